# revision 28
# baseline (speedup 1.0000x reference)
# Trainium2 Bass kernel for nn_ActionHead (Bahdanau additive attention +
# cross attention + projection head).
#
# Sharding: pure data-parallel over B — batch b runs on core b (B == 8 ==
# n_cores), weights replicated, no collectives.
#
# Per-core layout strategy: activations are kept transposed
# [D-on-partitions, tokens-on-free] so that
#   * the (N,P,D) additive-attention broadcast add M_proj[n,:]+O_proj[p,:]
#     rides the ACT engine's per-partition bias operand (one fused
#     tanh(M_projT_chunk + O_col) instruction per (d-chunk, p)),
#   * the mean over D becomes a TensorEngine reduction (tanh tile as the
#     stationary operand x ones column) landing in an [n, p] scores tile,
#   * all torch-convention (in,out) weight matrices are consumed in their
#     natural layout as the stationary matmul operand.
# Matmuls run in bf16 with fp32 PSUM accumulation; linear-layer biases are
# injected as rank-1 (K=1) matmuls accumulated into PSUM; softmax
# denominators ride activation accum_out / PE ones-reductions + DVE
# reciprocal; rsqrt for layernorm/L2-normalize uses the magic-constant +
# Newton iteration on DVE so every ACT function stays inside the single
# "exp_and_others" table set (no activation-table swaps).

import numpy as np

import concourse.bass as bass
import concourse.mybir as mybir
import concourse.tile as tile
from concourse import bacc
from concourse.bass_utils import run_bass_kernel_spmd
from concourse.masks import make_identity

B, N, P, D = 8, 256, 64, 512
ACTION_DIM = 512
LN_EPS = 1e-5
NC = 8

F32 = mybir.dt.float32
BF16 = mybir.dt.bfloat16
U32 = mybir.dt.uint32
AX = mybir.AluOpType
ACTF = mybir.ActivationFunctionType

DC = D // 128          # 4 chunks of the embedding dim on partitions
NT = N // 128          # 2 chunks of the motion-token dim on partitions
KC_F = (2 * D) // 128  # 8 contraction chunks for the fusion matmul

MAGIC = 0x5F3759DF


def _rsqrt(nc, pool, t_f32, rows):
    """rsqrt(t) for a [rows,1] fp32 SBUF column via magic-constant + 3
    Newton steps, entirely on DVE (avoids ACT sqrt, which lives in a
    different activation-table set)."""
    y = pool.tile([rows, 1], F32, tag="rsq_y")
    half_t = pool.tile([rows, 1], F32, tag="rsq_h")
    tmp = pool.tile([rows, 1], F32, tag="rsq_t")
    magic = pool.tile([rows, 1], U32, tag="rsq_m")
    nc.vector.memset(magic, MAGIC)
    # y = bitcast(MAGIC - (bitcast(t) >> 1))
    nc.vector.tensor_scalar(y.bitcast(U32), t_f32.bitcast(U32), 1, None,
                            AX.logical_shift_right)
    nc.vector.tensor_tensor(y.bitcast(U32), magic, y.bitcast(U32), AX.subtract)
    nc.vector.tensor_scalar(half_t, t_f32, 0.5, None, AX.mult)
    for _ in range(2):
        # y <- y * (1.5 - 0.5*t*y*y)
        nc.vector.tensor_tensor(tmp, y, y, AX.mult)
        nc.vector.tensor_tensor(tmp, tmp, half_t, AX.mult)
        nc.vector.tensor_scalar(tmp, tmp, -1.0, 1.5, AX.mult, AX.add)
        nc.vector.tensor_tensor(y, y, tmp, AX.mult)
    return y


def build_nc(reps=1, loop_n=None):
    """reps>1 statically unrolls the whole body; loop_n wraps the body in a
    hardware For_i loop (both only used for slope-based timing — the graded
    path is reps=1, loop_n=None)."""
    nc = bacc.Bacc("TRN2", enable_partition_id=False)

    mot = nc.dram_tensor("motion", [N, D], F32, kind="ExternalInput")
    obj = nc.dram_tensor("object", [P, D], F32, kind="ExternalInput")
    w_alpha = nc.dram_tensor("W_alpha", [D, D], F32, kind="ExternalInput")
    u_alpha = nc.dram_tensor("U_alpha", [D, D], F32, kind="ExternalInput")
    wq = nc.dram_tensor("Wq", [D, D], F32, kind="ExternalInput")
    wk = nc.dram_tensor("Wk", [D, D], F32, kind="ExternalInput")
    wv = nc.dram_tensor("Wv", [D, D], F32, kind="ExternalInput")
    wf = nc.dram_tensor("Wf", [2 * D, D], F32, kind="ExternalInput")
    wfc = nc.dram_tensor("Wfc", [D, ACTION_DIM], F32, kind="ExternalInput")
    b_alpha = nc.dram_tensor("b_alpha", [1, D], F32, kind="ExternalInput")
    bq = nc.dram_tensor("bq", [1, D], F32, kind="ExternalInput")
    bk = nc.dram_tensor("bk", [1, D], F32, kind="ExternalInput")
    bv = nc.dram_tensor("bv", [1, D], F32, kind="ExternalInput")
    bf_b = nc.dram_tensor("bf", [1, D], F32, kind="ExternalInput")
    ln_g = nc.dram_tensor("ln_g", [1, D], F32, kind="ExternalInput")
    ln_b = nc.dram_tensor("ln_b", [1, D], F32, kind="ExternalInput")
    bfc = nc.dram_tensor("bfc", [1, ACTION_DIM], F32, kind="ExternalInput")
    attn_out = nc.dram_tensor("attn_out", [P, D], F32, kind="ExternalOutput")
    projected = nc.dram_tensor("projected", [P, ACTION_DIM], F32,
                               kind="ExternalOutput")

    with tile.TileContext(nc) as tc:
        with (
            tc.tile_pool(name="consts", bufs=1) as consts,
            tc.tile_pool(name="weights", bufs=1) as wpool,
            tc.tile_pool(name="wstage", bufs=6) as wstage_pool,
            tc.tile_pool(name="acts", bufs=1) as acts,
            tc.tile_pool(name="tanh", bufs=6) as tanh_pool,
            tc.tile_pool(name="small", bufs=4) as small,
        ):
            def emit_body():
                # ---- constants --------------------------------------------
                ident = consts.tile([128, 128], BF16, tag="ident")
                make_identity(nc, ident)
                ones_r128_bf = consts.tile([1, 128], BF16, tag="o1")
                nc.vector.memset(ones_r128_bf, 1.0)
                ones_r128_f32 = consts.tile([1, 128], F32, tag="o2")
                nc.vector.memset(ones_r128_f32, 1.0)
                ones_rp_bf = consts.tile([1, P], BF16, tag="o3")
                nc.vector.memset(ones_rp_bf, 1.0)
                ones_rn_bf = consts.tile([1, N], BF16, tag="o4")
                nc.vector.memset(ones_rn_bf, 1.0)
                ones_c128_bf = consts.tile([128, 1], BF16, tag="o5")
                nc.vector.memset(ones_c128_bf, 1.0)
                ones96 = consts.tile([96, 128], BF16, tag="o6")
                nc.vector.memset(ones96, 1.0)

                tr_cm = tc.tile_pool(name="tr_psum", bufs=2, space="PSUM")
                tr_psum = tr_cm.__enter__()

                # ---- weights: rotating fp32 staging -> bf16 (Bacc's
                # event-semaphore pass legalizes the multi-wait DMAs) -------
                def load_weight_bf(name, dram, kchunks, free, engine_alt):
                    t = wpool.tile([128, kchunks * free], BF16, tag=f"w_{name}")
                    for kc in range(kchunks):
                        sl = slice(kc * free, (kc + 1) * free)
                        st = wstage_pool.tile([128, free], F32, tag="wstage")
                        nc.sync.dma_start(st,
                                          dram[kc * 128:(kc + 1) * 128, :])
                        eng = nc.gpsimd if engine_alt else nc.vector
                        eng.tensor_copy(t[:, sl], st)
                    return t

                wa_bf = load_weight_bf("wa", w_alpha, DC, D, False)
                ua_bf = load_weight_bf("ua", u_alpha, DC, D, False)

                row_stage = consts.tile([1, 10 * D], F32, tag="rows")
                _row_off = [0]

                def load_row_f32(dram, width):
                    o = _row_off[0]
                    _row_off[0] += width
                    st = row_stage[:, o:o + width]
                    nc.sync.dma_start(st, dram[:, :])
                    return st

                def load_row_bf(dram, width):
                    st = load_row_f32(dram, width)
                    r = consts.tile([1, width], BF16, tag=f"row_{dram.name}")
                    nc.vector.tensor_copy(r, st)
                    return r

                ba_row = load_row_bf(b_alpha, D)

                # ln scale/shift replicated over partitions via ones-matmul
                def replicate_row(dram, pool):
                    st = load_row_f32(dram, D)
                    ps = pool.tile([128, D], F32, tag="mm")
                    nc.tensor.matmul(ps, ones_r128_f32, st, start=True,
                                     stop=True)
                    sb = consts.tile([128, D], BF16, tag=f"rep_{dram.name}")
                    nc.vector.tensor_copy(sb, ps)
                    return sb


                # ---- motion / object loads + transposes -------------------
                mot_nat = acts.tile([128, NT * D], BF16, tag="motn")
                mot_st = acts.tile([128, NT * D], F32, tag="mots")
                for nt in range(NT):
                    sl = slice(nt * D, (nt + 1) * D)
                    nc.sync.dma_start(mot_st[:, sl],
                                      mot[nt * 128:(nt + 1) * 128, :])
                    nc.vector.tensor_copy(mot_nat[:, sl], mot_st[:, sl])
                obj_nat = acts.tile([P, D], BF16, tag="objn")
                obj_st = acts.tile([P, D], F32, tag="objs")
                nc.sync.dma_start(obj_st, obj[:, :])
                nc.vector.tensor_copy(obj_nat, obj_st)

                motT = acts.tile([128, DC * N], BF16, tag="motT")
                for nt in range(NT):
                    for dc in range(DC):
                        pt = tr_psum.tile([128, 128], BF16, tag="tr")
                        nc.tensor.transpose(
                            pt,
                            mot_nat[:, nt * D + dc * 128: nt * D + (dc + 1) * 128],
                            ident)
                        nc.vector.tensor_copy(
                            motT[:, dc * N + nt * 128: dc * N + nt * 128 + 128],
                            pt)
                objT = acts.tile([128, DC * P], BF16, tag="objT")
                for dc in range(DC):
                    pt = tr_psum.tile([128, P], BF16, tag="tr")
                    nc.tensor.transpose(
                        pt, obj_nat[:, dc * 128:(dc + 1) * 128], ident[:P, :P])
                    nc.vector.tensor_copy(objT[:, dc * P:(dc + 1) * P], pt)

                # ---- additive attention prep ------------------------------
                # O_proj natural [p, d'] (+ b_alpha via rank-1), then the
                # per-lane delta rows d4[p] = O[p] - O[p-4] (d4[p<4] = O[p])
                # that drive the PE rank-1 accumulation chains.
                onat_ps = tr_psum.tile([P, D], F32, tag="tr")
                for kc in range(DC):
                    nc.tensor.matmul(
                        onat_ps, objT[:, kc * P:(kc + 1) * P],
                        ua_bf[:, kc * D:(kc + 1) * D],
                        start=(kc == 0), stop=(kc == DC - 1))
                nc.tensor.matmul(onat_ps, ones_rp_bf[:, :P], ba_row,
                                 start=False, stop=True, skip_group_check=True)
                o_nat = acts.tile([P, D], F32, tag="onat")
                nc.vector.tensor_copy(o_nat, onat_ps)
                o_shift = acts.tile([P, D], F32, tag="osh")
                nc.vector.memset(o_shift[:4, :], 0.0)
                nc.sync.dma_start(o_shift[4:, :], o_nat[:P - 4, :])
                d4_bf = acts.tile([P, D], BF16, tag="d4")
                nc.vector.tensor_tensor(d4_bf, o_nat, o_shift, AX.subtract)
                # matmul operands must start at partition 0/32/64: flatten
                # the delta rows onto one partition so rhs slices are legal
                # delta rows flattened onto partitions {0,32,64} (the legal
                # matmul operand bases) so the flatten DMAs spread over three
                # SBUF ports instead of throttling through one
                d4_flat = acts.tile([96, 6 * 4 * D], BF16, tag="d4f")

                def _d4_slot(t):
                    seg = min(t // 6, 2)
                    return 32 * seg, (t - seg * 6) * 4 * D

                for t in range(P // 4):
                    row, col = _d4_slot(t)
                    nc.sync.dma_start(
                        d4_flat[row:row + 1, col:col + 4 * D].rearrange(
                            "o (p d) -> o p d", p=4),
                        d4_bf[t * 4:(t + 1) * 4, :])

                # ---- additive attention: tanh + mean over D ---------------
                # Two PSUM groups (one per n-tile) of 4 banks; bank j holds
                # M_proj[nt] + running sum of O rows for lane p = 4t + j.
                # Per step: 4 PE rank-1 delta matmuls advance the group, one
                # wide [128, 4*512] ACT tanh evaluates it, and a multi-dim
                # free-axis reduce_sum (alternating DVE / GPSIMD) produces
                # the 4 score columns. 32 ACT instructions total.
                tr_cm.__exit__(None, None, None)
                psT_cm = tc.tile_pool(name="psT", bufs=1, space="PSUM")
                psT = psT_cm.__enter__()
                groups = []
                for nt in range(NT):
                    g = psT.tile([128, DC * D], F32, tag=f"grp{nt}")
                    for j in range(DC):
                        for kc in range(DC):
                            nc.tensor.matmul(
                                g[:, j * D:(j + 1) * D],
                                motT[:, kc * N + nt * 128:
                                     kc * N + nt * 128 + 128],
                                wa_bf[:, kc * D:(kc + 1) * D],
                                start=(kc == 0), stop=(kc == DC - 1))
                    groups.append(g)

                scores_sb = acts.tile([128, NT * P], F32, tag="scores")
                n_steps = P // 4
                for t in range(n_steps):
                    for nt in range(NT):
                        g = groups[nt]
                        row, col = _d4_slot(t)
                        for j in range(4):
                            nc.tensor.matmul(
                                g[:, j * D:(j + 1) * D],
                                ones96[row:row + 1, :],
                                d4_flat[row:row + 1,
                                        col + j * D:col + (j + 1) * D],
                                start=False, stop=True,
                                skip_group_check=True)
                        th = tanh_pool.tile([128, 4, D], BF16, tag="th")
                        nc.scalar.activation(
                            th.rearrange("a b c -> a (b c)"),
                            g[:, :], ACTF.Tanh)
                        nc.vector.reduce_sum(
                            scores_sb[:, nt * P + t * 4: nt * P + t * 4 + 4],
                            th, axis=mybir.AxisListType.X)
                psT_cm.__exit__(None, None, None)

                # ---- late loads (overlap the tanh phase) ------------------
                wq_bf = load_weight_bf("wq", wq, DC, D, True)
                wk_bf = load_weight_bf("wk", wk, DC, D, True)
                wv_bf = load_weight_bf("wv", wv, DC, D, True)
                wf_bf = load_weight_bf("wf", wf, KC_F, D, True)
                wfc_bf = load_weight_bf("wfc", wfc, DC, ACTION_DIM, True)
                bq_row = load_row_bf(bq, D)
                bk_row = load_row_bf(bk, D)
                bv_row = load_row_bf(bv, D)
                bf_row = load_row_bf(bf_b, D)
                bfc_row = load_row_bf(bfc, ACTION_DIM)
                mm_cm = tc.tile_pool(name="mm_psum", bufs=6, space="PSUM")
                mm_psum = mm_cm.__enter__()
                g_rep = replicate_row(ln_g, mm_psum)
                b_rep = replicate_row(ln_b, mm_psum)

                # softmax over p (free axis; tanh-mean scores are in [-1,1]
                # so exp without max subtraction is safe; the 1/D mean is
                # folded into the activation scale). accum_out = denominator.
                w_sm = acts.tile([128, NT * P], BF16, tag="wsm")
                wT_sb = acts.tile([P, N], BF16, tag="wT")
                for nt in range(NT):
                    e_nt = tanh_pool.tile([128, P], BF16, tag="expnt")
                    den = small.tile([128, 1], F32, tag="den")
                    nc.scalar.activation(e_nt,
                                         scores_sb[:, nt * P:(nt + 1) * P],
                                         ACTF.Exp, scale=1.0 / D,
                                         accum_out=den)
                    denr = small.tile([128, 1], F32, tag="denr")
                    nc.vector.reciprocal(denr, den)
                    nc.vector.tensor_scalar(w_sm[:, nt * P:(nt + 1) * P],
                                            e_nt, denr, None, AX.mult)
                    pt = mm_psum.tile([P, 128], BF16, tag="mm")
                    nc.tensor.transpose(pt, w_sm[:, nt * P:(nt + 1) * P],
                                        ident)
                    nc.vector.tensor_copy(wT_sb[:, nt * 128:(nt + 1) * 128],
                                          pt)

                # M_e^T[d, n] = sum_p obj[p, d] * wT[p, n]
                meT = acts.tile([128, DC * N], BF16, tag="meT")
                for dc in range(DC):
                    mps = mm_psum.tile([128, N], F32, tag="mm")
                    nc.tensor.matmul(mps, obj_nat[:, dc * 128:(dc + 1) * 128],
                                     wT_sb, start=True, stop=True)
                    nc.vector.tensor_copy(meT[:, dc * N:(dc + 1) * N], mps)

                # ---- fuse: Mc = [motion, M_e] @ Wf + bf, then layernorm ----
                mc_bf = acts.tile([128, NT * D], BF16, tag="mcbf")
                for nt in range(NT):
                    mc_ps = mm_psum.tile([128, D], F32, tag="mm")
                    for kc in range(KC_F):
                        if kc < DC:
                            lhsT = motT[:, kc * N + nt * 128:
                                        kc * N + nt * 128 + 128]
                        else:
                            c = kc - DC
                            lhsT = meT[:, c * N + nt * 128:
                                       c * N + nt * 128 + 128]
                        nc.tensor.matmul(mc_ps, lhsT,
                                         wf_bf[:, kc * D:(kc + 1) * D],
                                         start=(kc == 0),
                                         stop=(kc == KC_F - 1))
                    nc.tensor.matmul(mc_ps, ones_r128_bf, bf_row,
                                     start=False, stop=True,
                                     skip_group_check=True)
                    # layernorm over free axis e
                    ssum = small.tile([128, 1], F32, tag="lnsum")
                    nc.vector.reduce_sum(ssum, mc_ps, axis=mybir.AxisListType.X)
                    negmean = small.tile([128, 1], F32, tag="lnm")
                    nc.vector.tensor_scalar(negmean, ssum, -1.0 / D, None,
                                            AX.mult)
                    sq_scr = tanh_pool.tile([128, D], BF16, tag="lnsq")
                    varsum = small.tile([128, 1], F32, tag="lnvs")
                    nc.scalar.activation(sq_scr, mc_ps, ACTF.Square,
                                         bias=negmean, accum_out=varsum)
                    varep = small.tile([128, 1], F32, tag="lnve")
                    nc.vector.tensor_scalar(varep, varsum, 1.0 / D, LN_EPS,
                                            AX.mult, AX.add)
                    rstd = _rsqrt(nc, small, varep, 128)
                    nmrs = small.tile([128, 1], F32, tag="lnnm")
                    nc.vector.tensor_tensor(nmrs, negmean, rstd, AX.mult)
                    ln1 = tanh_pool.tile([128, D], BF16, tag="ln1")
                    nc.scalar.activation(ln1, mc_ps, ACTF.Identity,
                                         bias=nmrs, scale=rstd)
                    # * g + b (free-axis affine, replicated tiles)
                    dst = mc_bf[:, nt * D:(nt + 1) * D]
                    nc.vector.tensor_tensor(dst, ln1, g_rep, AX.mult)
                    nc.vector.tensor_tensor(dst, dst, b_rep, AX.add)

                # transpose Mc -> McT [e-part, n-free]
                mcT = acts.tile([128, DC * N], BF16, tag="mcT")
                for nt in range(NT):
                    for dc in range(DC):
                        pt = mm_psum.tile([128, 128], BF16, tag="mm")
                        nc.tensor.transpose(
                            pt,
                            mc_bf[:, nt * D + dc * 128: nt * D + (dc + 1) * 128],
                            ident)
                        nc.vector.tensor_copy(
                            mcT[:, dc * N + nt * 128: dc * N + nt * 128 + 128],
                            pt)

                # ---- cross attention --------------------------------------
                kT = acts.tile([128, DC * N], BF16, tag="kT")
                for mc in range(DC):
                    kps = mm_psum.tile([128, N], F32, tag="mm")
                    for kc in range(DC):
                        nc.tensor.matmul(
                            kps,
                            wk_bf[:, kc * D + mc * 128: kc * D + (mc + 1) * 128],
                            mcT[:, kc * N:(kc + 1) * N],
                            start=(kc == 0), stop=(kc == DC - 1))
                    nc.tensor.matmul(kps, bk_row[:, mc * 128:(mc + 1) * 128],
                                     ones_rn_bf, start=False, stop=True,
                                     skip_group_check=True)
                    nc.vector.tensor_copy(kT[:, mc * N:(mc + 1) * N], kps)
                qT = acts.tile([128, DC * P], BF16, tag="qT")
                for mc in range(DC):
                    qps = mm_psum.tile([128, P], F32, tag="mm")
                    for kc in range(DC):
                        nc.tensor.matmul(
                            qps,
                            wq_bf[:, kc * D + mc * 128: kc * D + (mc + 1) * 128],
                            objT[:, kc * P:(kc + 1) * P],
                            start=(kc == 0), stop=(kc == DC - 1))
                    nc.tensor.matmul(qps, bq_row[:, mc * 128:(mc + 1) * 128],
                                     ones_rp_bf, start=False, stop=True,
                                     skip_group_check=True)
                    nc.vector.tensor_copy(qT[:, mc * P:(mc + 1) * P], qps)
                # V[n, d'] = McT^T Wv (+bv)
                v_bf = acts.tile([128, NT * D], BF16, tag="vbf")
                for nt in range(NT):
                    vps = mm_psum.tile([128, D], F32, tag="mm")
                    for kc in range(DC):
                        nc.tensor.matmul(
                            vps,
                            mcT[:, kc * N + nt * 128: kc * N + nt * 128 + 128],
                            wv_bf[:, kc * D:(kc + 1) * D],
                            start=(kc == 0), stop=(kc == DC - 1))
                    nc.tensor.matmul(vps, ones_r128_bf, bv_row,
                                     start=False, stop=True,
                                     skip_group_check=True)
                    nc.vector.tensor_copy(v_bf[:, nt * D:(nt + 1) * D], vps)

                # scores2^T[n, p] = K^T(slice)^T @ Q^T ; softmax over n
                e2_sb = acts.tile([128, NT * P], BF16, tag="e2")
                den2_ps = mm_psum.tile([1, P], F32, tag="mm")
                for nt in range(NT):
                    s2ps = mm_psum.tile([128, P], F32, tag="mm")
                    for kc in range(DC):
                        nc.tensor.matmul(
                            s2ps,
                            kT[:, kc * N + nt * 128: kc * N + nt * 128 + 128],
                            qT[:, kc * P:(kc + 1) * P],
                            start=(kc == 0), stop=(kc == DC - 1))
                    nc.scalar.activation(e2_sb[:, nt * P:(nt + 1) * P], s2ps,
                                         ACTF.Exp,
                                         scale=1.0 / float(np.sqrt(D)))
                    nc.tensor.matmul(den2_ps, ones_c128_bf,
                                     e2_sb[:, nt * P:(nt + 1) * P],
                                     start=(nt == 0), stop=(nt == NT - 1))
                den2r = small.tile([1, P], F32, tag="den2r")
                nc.vector.reciprocal(den2r, den2_ps)
                d2rep_ps = mm_psum.tile([128, P], F32, tag="mm")
                nc.tensor.matmul(d2rep_ps, ones_r128_f32, den2r, start=True,
                                 stop=True)
                w2T = acts.tile([128, NT * P], BF16, tag="w2T")
                for nt in range(NT):
                    nc.vector.tensor_tensor(
                        w2T[:, nt * P:(nt + 1) * P],
                        e2_sb[:, nt * P:(nt + 1) * P], d2rep_ps, AX.mult)

                # attn_output[p, d] = w2T^T @ V   (output #1)
                ao_ps = mm_psum.tile([P, D], F32, tag="mm")
                for nt in range(NT):
                    nc.tensor.matmul(ao_ps, w2T[:, nt * P:(nt + 1) * P],
                                     v_bf[:, nt * D:(nt + 1) * D],
                                     start=(nt == 0), stop=(nt == NT - 1))
                ao_sb = acts.tile([P, D], F32, tag="aosb")
                nc.vector.tensor_copy(ao_sb, ao_ps)
                nc.sync.dma_start(attn_out[:, :], ao_sb)

                # attn_output^T[d, p] for the final projection
                aoT_ps = mm_psum.tile([128, DC * P], F32, tag="mm")
                for dc in range(DC):
                    for nt in range(NT):
                        nc.tensor.matmul(
                            aoT_ps[:, dc * P:(dc + 1) * P],
                            v_bf[:, nt * D + dc * 128: nt * D + (dc + 1) * 128],
                            w2T[:, nt * P:(nt + 1) * P],
                            start=(nt == 0), stop=(nt == NT - 1))
                aoT = acts.tile([128, DC * P], BF16, tag="aoT")
                nc.vector.tensor_copy(aoT, aoT_ps)

                # projected[p, a] = aoT^T @ Wfc + bfc, L2-normalize rows
                pr_ps = mm_psum.tile([P, ACTION_DIM], F32, tag="mm")
                for dc in range(DC):
                    nc.tensor.matmul(
                        pr_ps, aoT[:, dc * P:(dc + 1) * P],
                        wfc_bf[:, dc * ACTION_DIM:(dc + 1) * ACTION_DIM],
                        start=(dc == 0), stop=(dc == DC - 1))
                nc.tensor.matmul(pr_ps, ones_rp_bf, bfc_row,
                                 start=False, stop=True, skip_group_check=True)
                sq2 = tanh_pool.tile([P, ACTION_DIM], BF16, tag="l2sq")
                ss = small.tile([P, 1], F32, tag="l2ss")
                nc.scalar.activation(sq2, pr_ps, ACTF.Square, accum_out=ss)
                rn = _rsqrt(nc, small, ss, P)
                # 1/max(||x||, 1e-12) == min(rsqrt(ss), 1e12)
                nc.vector.tensor_scalar(rn, rn, 1e12, None, AX.min)
                pr_sb = acts.tile([P, ACTION_DIM], F32, tag="prsb")
                nc.scalar.activation(pr_sb, pr_ps, ACTF.Identity, scale=rn)
                nc.sync.dma_start(projected[:, :], pr_sb)
                mm_cm.__exit__(None, None, None)

            if loop_n is not None:
                with tc.For_i(0, loop_n, 1):
                    emit_body()
            else:
                for _rep in range(reps):
                    emit_body()

    nc.finalize()
    return nc


_CACHED_NC = {}


def _get_nc(reps=1, loop_n=None):
    key = (reps, loop_n)
    if key not in _CACHED_NC:
        _CACHED_NC[key] = build_nc(reps, loop_n)
    return _CACHED_NC[key]


def _make_in_maps(inputs):
    f = np.float32

    def arr(x):
        return np.ascontiguousarray(np.asarray(x, dtype=f))

    shared = {
        "W_alpha": arr(inputs["W_alpha"]), "U_alpha": arr(inputs["U_alpha"]),
        "Wq": arr(inputs["Wq"]), "Wk": arr(inputs["Wk"]), "Wv": arr(inputs["Wv"]),
        "Wf": arr(inputs["Wf"]), "Wfc": arr(inputs["Wfc"]),
        "b_alpha": arr(inputs["b_alpha"]).reshape(1, D),
        "bq": arr(inputs["bq"]).reshape(1, D),
        "bk": arr(inputs["bk"]).reshape(1, D),
        "bv": arr(inputs["bv"]).reshape(1, D),
        "bf": arr(inputs["bf"]).reshape(1, D),
        "ln_g": arr(inputs["ln_g"]).reshape(1, D),
        "ln_b": arr(inputs["ln_b"]).reshape(1, D),
        "bfc": arr(inputs["bfc"]).reshape(1, ACTION_DIM),
    }
    motion = arr(inputs["motion_features"])
    objf = arr(inputs["object_features"])
    return [
        {"motion": np.ascontiguousarray(motion[c]),
         "object": np.ascontiguousarray(objf[c]), **shared}
        for c in range(NC)
    ]


def _run(inputs, trace=False):
    nc = _get_nc()
    in_maps = _make_in_maps(inputs)
    res = run_bass_kernel_spmd(nc, in_maps, core_ids=list(range(NC)),
                               trace=trace)
    attn = np.stack([r["attn_out"] for r in res.results])
    proj = np.stack([r["projected"] for r in res.results])
    return (attn, proj), res


def kernel(**inputs):
    (attn, proj), _ = _run(inputs)
    return attn, proj


def bench(inputs, loops=(4, 36)):
    """Time the kernel body on device: build two NEFFs whose body runs in a
    hardware For_i loop loops[0] / loops[1] times, measure pipelined wall
    time for each, return the per-iteration slope in ns (cancels constant
    axon dispatch overhead)."""
    import time

    import jax
    from jax.experimental.shard_map import shard_map
    from jax.sharding import Mesh, PartitionSpec, NamedSharding
    import concourse.mybir as mb
    from concourse.bass2jax import _bass_exec_p, install_neuronx_cc_hook

    install_neuronx_cc_hook()
    in_maps = _make_in_maps(inputs)
    nc0 = _get_nc(1, loops[0])

    in_names, out_names, out_avals, zero_outs = [], [], [], []
    for alloc in nc0.m.functions[0].allocations:
        if not isinstance(alloc, mb.MemoryLocationSet):
            continue
        name = alloc.memorylocations[0].name
        if alloc.kind == "ExternalInput":
            in_names.append(name)
        elif alloc.kind == "ExternalOutput":
            shape = tuple(alloc.tensor_shape)
            dtype = mb.dt.np(alloc.dtype)
            out_names.append(name)
            out_avals.append(jax.core.ShapedArray(shape, dtype))
            zero_outs.append(np.zeros(shape, dtype))
    n_params = len(in_names)
    all_names = in_names + out_names

    devices = jax.devices()[:NC]
    mesh = Mesh(np.asarray(devices), ("core",))
    spec = PartitionSpec("core")
    in_specs = (spec,) * (n_params + len(out_names))
    out_specs = (spec,) * len(out_names)
    sharding = NamedSharding(mesh, spec)
    concat_in = [
        jax.device_put(
            np.concatenate([np.asarray(in_maps[c][n]) for c in range(NC)],
                           axis=0), sharding)
        for n in in_names
    ]
    concat_zero = [
        jax.device_put(np.zeros((NC * z.shape[0], *z.shape[1:]), z.dtype),
                       sharding)
        for z in zero_outs
    ]

    def make_fn(loop_n):
        nck = _get_nc(1, loop_n)

        def _bodyk(*args):
            outs = _bass_exec_p.bind(
                *args,
                out_avals=tuple(out_avals),
                in_names=tuple(all_names),
                out_names=tuple(out_names),
                lowering_input_output_aliases=(),
                sim_require_finite=True,
                sim_require_nnan=True,
                nc=nck,
            )
            return tuple(outs)

        fn = jax.jit(shard_map(_bodyk, mesh=mesh, in_specs=in_specs,
                               out_specs=out_specs, check_rep=False),
                     keep_unused=True)
        jax.block_until_ready(fn(*concat_in, *concat_zero))
        return fn

    fns = {k: make_fn(k) for k in loops}

    def timed(fn, iters=16):
        t0 = time.perf_counter()
        outs = [fn(*concat_in, *concat_zero) for _ in range(iters)]
        jax.block_until_ready(outs)
        return (time.perf_counter() - t0) / iters

    # interleave measurement rounds so slow drift cancels
    best = {k: None for k in loops}
    for _ in range(6):
        for k in loops:
            dt = timed(fns[k])
            best[k] = dt if best[k] is None else min(best[k], dt)
    k0, k1 = loops
    per_iter = (best[k1] - best[k0]) / (k1 - k0)
    print(f"bench: t{k0}={best[k0]*1e6:.1f}us  t{k1}={best[k1]*1e6:.1f}us  "
          f"slope={per_iter*1e6:.2f}us/iter")
    return per_iter * 1e9


# revision 30
# speedup vs baseline: 1.3869x; 1.3869x over previous
# Trainium2 Bass kernel for nn_ActionHead (Bahdanau additive attention +
# cross attention + projection head).
#
# Sharding: pure data-parallel over B — batch b runs on core b (B == 8 ==
# n_cores), weights replicated, no collectives.
#
# Per-core layout strategy: activations are kept transposed
# [D-on-partitions, tokens-on-free] so that
#   * the (N,P,D) additive-attention broadcast add M_proj[n,:]+O_proj[p,:]
#     rides the ACT engine's per-partition bias operand (one fused
#     tanh(M_projT_chunk + O_col) instruction per (d-chunk, p)),
#   * the mean over D becomes a TensorEngine reduction (tanh tile as the
#     stationary operand x ones column) landing in an [n, p] scores tile,
#   * all torch-convention (in,out) weight matrices are consumed in their
#     natural layout as the stationary matmul operand.
# Matmuls run in bf16 with fp32 PSUM accumulation; linear-layer biases are
# injected as rank-1 (K=1) matmuls accumulated into PSUM; softmax
# denominators ride activation accum_out / PE ones-reductions + DVE
# reciprocal; rsqrt for layernorm/L2-normalize uses the magic-constant +
# Newton iteration on DVE so every ACT function stays inside the single
# "exp_and_others" table set (no activation-table swaps).

import numpy as np

import concourse.bass as bass
import concourse.mybir as mybir
import concourse.tile as tile
from concourse import bacc
from concourse.bass_utils import run_bass_kernel_spmd
from concourse.masks import make_identity

B, N, P, D = 8, 256, 64, 512
ACTION_DIM = 512
LN_EPS = 1e-5
NC = 8

F32 = mybir.dt.float32
BF16 = mybir.dt.bfloat16
U32 = mybir.dt.uint32
AX = mybir.AluOpType
ACTF = mybir.ActivationFunctionType

DC = D // 128          # 4 chunks of the embedding dim on partitions
NT = N // 128          # 2 chunks of the motion-token dim on partitions
KC_F = (2 * D) // 128  # 8 contraction chunks for the fusion matmul

MAGIC = 0x5F3759DF


def _rsqrt(nc, pool, t_f32, rows):
    """rsqrt(t) for a [rows,1] fp32 SBUF column via magic-constant + 3
    Newton steps, entirely on DVE (avoids ACT sqrt, which lives in a
    different activation-table set)."""
    y = pool.tile([rows, 1], F32, tag="rsq_y")
    half_t = pool.tile([rows, 1], F32, tag="rsq_h")
    tmp = pool.tile([rows, 1], F32, tag="rsq_t")
    magic = pool.tile([rows, 1], U32, tag="rsq_m")
    nc.vector.memset(magic, MAGIC)
    # y = bitcast(MAGIC - (bitcast(t) >> 1))
    nc.vector.tensor_scalar(y.bitcast(U32), t_f32.bitcast(U32), 1, None,
                            AX.logical_shift_right)
    nc.vector.tensor_tensor(y.bitcast(U32), magic, y.bitcast(U32), AX.subtract)
    nc.vector.tensor_scalar(half_t, t_f32, 0.5, None, AX.mult)
    for _ in range(2):
        # y <- y * (1.5 - 0.5*t*y*y)
        nc.vector.tensor_tensor(tmp, y, y, AX.mult)
        nc.vector.tensor_tensor(tmp, tmp, half_t, AX.mult)
        nc.vector.tensor_scalar(tmp, tmp, -1.0, 1.5, AX.mult, AX.add)
        nc.vector.tensor_tensor(y, y, tmp, AX.mult)
    return y


def build_nc(reps=1, loop_n=None):
    """reps>1 statically unrolls the whole body; loop_n wraps the body in a
    hardware For_i loop (both only used for slope-based timing — the graded
    path is reps=1, loop_n=None)."""
    nc = bacc.Bacc("TRN2", enable_partition_id=False)

    mot = nc.dram_tensor("motion", [N, D], F32, kind="ExternalInput")
    obj = nc.dram_tensor("object", [P, D], F32, kind="ExternalInput")
    w_alpha = nc.dram_tensor("W_alpha", [D, D], F32, kind="ExternalInput")
    u_alpha = nc.dram_tensor("U_alpha", [D, D], F32, kind="ExternalInput")
    wq = nc.dram_tensor("Wq", [D, D], F32, kind="ExternalInput")
    wk = nc.dram_tensor("Wk", [D, D], F32, kind="ExternalInput")
    wv = nc.dram_tensor("Wv", [D, D], F32, kind="ExternalInput")
    wf = nc.dram_tensor("Wf", [2 * D, D], F32, kind="ExternalInput")
    wfc = nc.dram_tensor("Wfc", [D, ACTION_DIM], F32, kind="ExternalInput")
    b_alpha = nc.dram_tensor("b_alpha", [1, D], F32, kind="ExternalInput")
    bq = nc.dram_tensor("bq", [1, D], F32, kind="ExternalInput")
    bk = nc.dram_tensor("bk", [1, D], F32, kind="ExternalInput")
    bv = nc.dram_tensor("bv", [1, D], F32, kind="ExternalInput")
    bf_b = nc.dram_tensor("bf", [1, D], F32, kind="ExternalInput")
    ln_g = nc.dram_tensor("ln_g", [1, D], F32, kind="ExternalInput")
    ln_b = nc.dram_tensor("ln_b", [1, D], F32, kind="ExternalInput")
    bfc = nc.dram_tensor("bfc", [1, ACTION_DIM], F32, kind="ExternalInput")
    attn_out = nc.dram_tensor("attn_out", [P, D], F32, kind="ExternalOutput")
    projected = nc.dram_tensor("projected", [P, ACTION_DIM], F32,
                               kind="ExternalOutput")

    with tile.TileContext(nc) as tc:
        with (
            tc.tile_pool(name="consts", bufs=1) as consts,
            tc.tile_pool(name="weights", bufs=1) as wpool,
            tc.tile_pool(name="wstage", bufs=4) as wstage_pool,
            tc.tile_pool(name="acts", bufs=1) as acts,
            tc.tile_pool(name="tanh", bufs=4) as tanh_pool,
            tc.tile_pool(name="small", bufs=4) as small,
        ):
            def emit_body():
                # ---- constants --------------------------------------------
                ident = consts.tile([128, 128], BF16, tag="ident")
                make_identity(nc, ident)
                ones_r128_bf = consts.tile([1, 128], BF16, tag="o1")
                nc.vector.memset(ones_r128_bf, 1.0)
                ones_r128_f32 = consts.tile([1, 128], F32, tag="o2")
                nc.vector.memset(ones_r128_f32, 1.0)
                ones_rp_bf = consts.tile([1, P], BF16, tag="o3")
                nc.vector.memset(ones_rp_bf, 1.0)
                ones_rn_bf = consts.tile([1, N], BF16, tag="o4")
                nc.vector.memset(ones_rn_bf, 1.0)
                ones_c128_bf = consts.tile([128, 1], BF16, tag="o5")
                nc.vector.memset(ones_c128_bf, 1.0)
                ones96 = consts.tile([96, 128], BF16, tag="o6")
                nc.vector.memset(ones96, 1.0)

                tr_cm = tc.tile_pool(name="tr_psum", bufs=2, space="PSUM")
                tr_psum = tr_cm.__enter__()

                # ---- weights: rotating fp32 staging -> bf16 (Bacc's
                # event-semaphore pass legalizes the multi-wait DMAs) -------
                def load_weight_bf(name, dram, kchunks, free, engine_alt):
                    t = wpool.tile([128, kchunks * free], BF16, tag=f"w_{name}")
                    for kc in range(kchunks):
                        sl = slice(kc * free, (kc + 1) * free)
                        st = wstage_pool.tile([128, free], F32, tag="wstage")
                        nc.sync.dma_start(st,
                                          dram[kc * 128:(kc + 1) * 128, :])
                        eng = nc.gpsimd if engine_alt else nc.vector
                        eng.tensor_copy(t[:, sl], st)
                    return t

                wa_bf = load_weight_bf("wa", w_alpha, DC, D, False)
                ua_bf = load_weight_bf("ua", u_alpha, DC, D, False)

                row_stage = consts.tile([1, 10 * D], F32, tag="rows")
                _row_off = [0]

                def load_row_f32(dram, width):
                    o = _row_off[0]
                    _row_off[0] += width
                    st = row_stage[:, o:o + width]
                    nc.sync.dma_start(st, dram[:, :])
                    return st

                def load_row_bf(dram, width):
                    st = load_row_f32(dram, width)
                    r = consts.tile([1, width], BF16, tag=f"row_{dram.name}")
                    nc.vector.tensor_copy(r, st)
                    return r

                ba_row = load_row_bf(b_alpha, D)

                # ln scale/shift replicated over partitions via ones-matmul
                def replicate_row(dram, pool):
                    st = load_row_f32(dram, D)
                    ps = pool.tile([128, D], F32, tag="mm")
                    nc.tensor.matmul(ps, ones_r128_f32, st, start=True,
                                     stop=True)
                    sb = consts.tile([128, D], BF16, tag=f"rep_{dram.name}")
                    nc.vector.tensor_copy(sb, ps)
                    return sb


                # ---- motion / object loads + transposes -------------------
                mot_nat = acts.tile([128, NT * D], BF16, tag="motn")
                mot_st = acts.tile([128, NT * D], F32, tag="mots")
                for nt in range(NT):
                    sl = slice(nt * D, (nt + 1) * D)
                    nc.sync.dma_start(mot_st[:, sl],
                                      mot[nt * 128:(nt + 1) * 128, :])
                    nc.vector.tensor_copy(mot_nat[:, sl], mot_st[:, sl])
                obj_nat = acts.tile([P, D], BF16, tag="objn")
                obj_st = acts.tile([P, D], F32, tag="objs")
                nc.sync.dma_start(obj_st, obj[:, :])
                nc.vector.tensor_copy(obj_nat, obj_st)

                motT = acts.tile([128, DC * N], BF16, tag="motT")
                for nt in range(NT):
                    for dc in range(DC):
                        pt = tr_psum.tile([128, 128], BF16, tag="tr")
                        nc.tensor.transpose(
                            pt,
                            mot_nat[:, nt * D + dc * 128: nt * D + (dc + 1) * 128],
                            ident)
                        nc.vector.tensor_copy(
                            motT[:, dc * N + nt * 128: dc * N + nt * 128 + 128],
                            pt)
                objT = acts.tile([128, DC * P], BF16, tag="objT")
                for dc in range(DC):
                    pt = tr_psum.tile([128, P], BF16, tag="tr")
                    nc.tensor.transpose(
                        pt, obj_nat[:, dc * 128:(dc + 1) * 128], ident[:P, :P])
                    nc.vector.tensor_copy(objT[:, dc * P:(dc + 1) * P], pt)

                # ---- additive attention prep ------------------------------
                # O_proj natural [p, d'] (+ b_alpha via rank-1), then the
                # per-lane delta rows d4[p] = O[p] - O[p-4] (d4[p<4] = O[p])
                # that drive the PE rank-1 accumulation chains.
                onat_ps = tr_psum.tile([P, D], F32, tag="tr")
                for kc in range(DC):
                    nc.tensor.matmul(
                        onat_ps, objT[:, kc * P:(kc + 1) * P],
                        ua_bf[:, kc * D:(kc + 1) * D],
                        start=(kc == 0), stop=(kc == DC - 1))
                nc.tensor.matmul(onat_ps, ones_rp_bf[:, :P], ba_row,
                                 start=False, stop=True, skip_group_check=True)
                o_nat = acts.tile([P, D], F32, tag="onat")
                nc.vector.tensor_copy(o_nat, onat_ps)
                o_shift = acts.tile([P, D], F32, tag="osh")
                nc.vector.memset(o_shift[:4, :], 0.0)
                nc.sync.dma_start(o_shift[4:, :], o_nat[:P - 4, :])
                d4_bf = acts.tile([P, D], BF16, tag="d4")
                nc.vector.tensor_tensor(d4_bf, o_nat, o_shift, AX.subtract)
                # matmul operands must start at partition 0/32/64: flatten
                # the delta rows onto one partition so rhs slices are legal
                # delta rows flattened onto partitions {0,32,64} (the legal
                # matmul operand bases) so the flatten DMAs spread over three
                # SBUF ports instead of throttling through one
                d4_flat = acts.tile([96, 6 * 4 * D], BF16, tag="d4f")

                def _d4_slot(t):
                    seg = min(t // 6, 2)
                    return 32 * seg, (t - seg * 6) * 4 * D

                for t in range(P // 4):
                    row, col = _d4_slot(t)
                    nc.sync.dma_start(
                        d4_flat[row:row + 1, col:col + 4 * D].rearrange(
                            "o (p d) -> o p d", p=4),
                        d4_bf[t * 4:(t + 1) * 4, :])

                # ---- additive attention: tanh + mean over D ---------------
                # Two PSUM groups (one per n-tile) of 4 banks; bank j holds
                # M_proj[nt] + running sum of O rows for lane p = 4t + j.
                # Per step: 4 PE rank-1 delta matmuls advance the group, one
                # wide [128, 4*512] ACT tanh evaluates it, and a multi-dim
                # free-axis reduce_sum (alternating DVE / GPSIMD) produces
                # the 4 score columns. 32 ACT instructions total.
                tr_cm.__exit__(None, None, None)
                psT_cm = tc.tile_pool(name="psT", bufs=1, space="PSUM")
                psT = psT_cm.__enter__()
                groups = []
                for nt in range(NT):
                    g = psT.tile([128, DC * D], F32, tag=f"grp{nt}")
                    for j in range(DC):
                        for kc in range(DC):
                            nc.tensor.matmul(
                                g[:, j * D:(j + 1) * D],
                                motT[:, kc * N + nt * 128:
                                     kc * N + nt * 128 + 128],
                                wa_bf[:, kc * D:(kc + 1) * D],
                                start=(kc == 0), stop=(kc == DC - 1))
                    groups.append(g)

                scores_sb = acts.tile([128, NT * P], F32, tag="scores")
                n_steps = P // 4
                for t in range(n_steps):
                    for nt in range(NT):
                        g = groups[nt]
                        row, col = _d4_slot(t)
                        for j in range(4):
                            nc.tensor.matmul(
                                g[:, j * D:(j + 1) * D],
                                ones96[row:row + 1, :],
                                d4_flat[row:row + 1,
                                        col + j * D:col + (j + 1) * D],
                                start=False, stop=True,
                                skip_group_check=True)
                        th = tanh_pool.tile([128, 4, D], BF16, tag="th")
                        nc.scalar.activation(
                            th.rearrange("a b c -> a (b c)"),
                            g[:, :], ACTF.Tanh)
                        nc.vector.reduce_sum(
                            scores_sb[:, nt * P + t * 4: nt * P + t * 4 + 4],
                            th, axis=mybir.AxisListType.X)
                psT_cm.__exit__(None, None, None)

                # ---- late loads (overlap the tanh phase) ------------------
                wq_bf = load_weight_bf("wq", wq, DC, D, True)
                wk_bf = load_weight_bf("wk", wk, DC, D, True)
                wv_bf = load_weight_bf("wv", wv, DC, D, True)
                wf_bf = load_weight_bf("wf", wf, KC_F, D, True)
                wfc_bf = load_weight_bf("wfc", wfc, DC, ACTION_DIM, True)
                bq_row = load_row_bf(bq, D)
                bk_row = load_row_bf(bk, D)
                bv_row = load_row_bf(bv, D)
                bf_row = load_row_bf(bf_b, D)
                bfc_row = load_row_bf(bfc, ACTION_DIM)
                mm_cm = tc.tile_pool(name="mm_psum", bufs=3, space="PSUM")
                mm_psum = mm_cm.__enter__()
                g_rep = replicate_row(ln_g, mm_psum)
                b_rep = replicate_row(ln_b, mm_psum)

                # softmax over p (free axis; tanh-mean scores are in [-1,1]
                # so exp without max subtraction is safe; the 1/D mean is
                # folded into the activation scale). accum_out = denominator.
                w_sm = acts.tile([128, NT * P], BF16, tag="wsm")
                wT_sb = acts.tile([P, N], BF16, tag="wT")
                for nt in range(NT):
                    e_nt = tanh_pool.tile([128, P], BF16, tag="expnt")
                    den = small.tile([128, 1], F32, tag="den")
                    nc.scalar.activation(e_nt,
                                         scores_sb[:, nt * P:(nt + 1) * P],
                                         ACTF.Exp, scale=1.0 / D,
                                         accum_out=den)
                    denr = small.tile([128, 1], F32, tag="denr")
                    nc.vector.reciprocal(denr, den)
                    nc.vector.tensor_scalar(w_sm[:, nt * P:(nt + 1) * P],
                                            e_nt, denr, None, AX.mult)
                    pt = mm_psum.tile([P, 128], BF16, tag="mm")
                    nc.tensor.transpose(pt, w_sm[:, nt * P:(nt + 1) * P],
                                        ident)
                    nc.vector.tensor_copy(wT_sb[:, nt * 128:(nt + 1) * 128],
                                          pt)

                # M_e^T[d, n] = sum_p obj[p, d] * wT[p, n]
                meT = acts.tile([128, DC * N], BF16, tag="meT")
                for dc in range(DC):
                    mps = mm_psum.tile([128, N], F32, tag="mm")
                    nc.tensor.matmul(mps, obj_nat[:, dc * 128:(dc + 1) * 128],
                                     wT_sb, start=True, stop=True)
                    nc.vector.tensor_copy(meT[:, dc * N:(dc + 1) * N], mps)

                # ---- fuse: Mc = [motion, M_e] @ Wf + bf, then layernorm ----
                mc_bf = acts.tile([128, NT * D], BF16, tag="mcbf")
                for nt in range(NT):
                    mc_ps = mm_psum.tile([128, D], F32, tag="mm")
                    for kc in range(KC_F):
                        if kc < DC:
                            lhsT = motT[:, kc * N + nt * 128:
                                        kc * N + nt * 128 + 128]
                        else:
                            c = kc - DC
                            lhsT = meT[:, c * N + nt * 128:
                                       c * N + nt * 128 + 128]
                        nc.tensor.matmul(mc_ps, lhsT,
                                         wf_bf[:, kc * D:(kc + 1) * D],
                                         start=(kc == 0),
                                         stop=(kc == KC_F - 1))
                    nc.tensor.matmul(mc_ps, ones_r128_bf, bf_row,
                                     start=False, stop=True,
                                     skip_group_check=True)
                    # layernorm over free axis e
                    ssum = small.tile([128, 1], F32, tag="lnsum")
                    nc.vector.reduce_sum(ssum, mc_ps, axis=mybir.AxisListType.X)
                    negmean = small.tile([128, 1], F32, tag="lnm")
                    nc.vector.tensor_scalar(negmean, ssum, -1.0 / D, None,
                                            AX.mult)
                    sq_scr = tanh_pool.tile([128, D], BF16, tag="lnsq")
                    varsum = small.tile([128, 1], F32, tag="lnvs")
                    nc.scalar.activation(sq_scr, mc_ps, ACTF.Square,
                                         bias=negmean, accum_out=varsum)
                    varep = small.tile([128, 1], F32, tag="lnve")
                    nc.vector.tensor_scalar(varep, varsum, 1.0 / D, LN_EPS,
                                            AX.mult, AX.add)
                    rstd = _rsqrt(nc, small, varep, 128)
                    nmrs = small.tile([128, 1], F32, tag="lnnm")
                    nc.vector.tensor_tensor(nmrs, negmean, rstd, AX.mult)
                    ln1 = tanh_pool.tile([128, D], BF16, tag="ln1")
                    nc.scalar.activation(ln1, mc_ps, ACTF.Identity,
                                         bias=nmrs, scale=rstd)
                    # * g + b (free-axis affine, replicated tiles)
                    dst = mc_bf[:, nt * D:(nt + 1) * D]
                    nc.vector.tensor_tensor(dst, ln1, g_rep, AX.mult)
                    nc.vector.tensor_tensor(dst, dst, b_rep, AX.add)

                # transpose Mc -> McT [e-part, n-free]
                mcT = acts.tile([128, DC * N], BF16, tag="mcT")
                for nt in range(NT):
                    for dc in range(DC):
                        pt = mm_psum.tile([128, 128], BF16, tag="mm")
                        nc.tensor.transpose(
                            pt,
                            mc_bf[:, nt * D + dc * 128: nt * D + (dc + 1) * 128],
                            ident)
                        nc.vector.tensor_copy(
                            mcT[:, dc * N + nt * 128: dc * N + nt * 128 + 128],
                            pt)

                # ---- cross attention --------------------------------------
                kT = acts.tile([128, DC * N], BF16, tag="kT")
                for mc in range(DC):
                    kps = mm_psum.tile([128, N], F32, tag="mm")
                    for kc in range(DC):
                        nc.tensor.matmul(
                            kps,
                            wk_bf[:, kc * D + mc * 128: kc * D + (mc + 1) * 128],
                            mcT[:, kc * N:(kc + 1) * N],
                            start=(kc == 0), stop=(kc == DC - 1))
                    nc.tensor.matmul(kps, bk_row[:, mc * 128:(mc + 1) * 128],
                                     ones_rn_bf, start=False, stop=True,
                                     skip_group_check=True)
                    nc.vector.tensor_copy(kT[:, mc * N:(mc + 1) * N], kps)
                qT = acts.tile([128, DC * P], BF16, tag="qT")
                for mc in range(DC):
                    qps = mm_psum.tile([128, P], F32, tag="mm")
                    for kc in range(DC):
                        nc.tensor.matmul(
                            qps,
                            wq_bf[:, kc * D + mc * 128: kc * D + (mc + 1) * 128],
                            objT[:, kc * P:(kc + 1) * P],
                            start=(kc == 0), stop=(kc == DC - 1))
                    nc.tensor.matmul(qps, bq_row[:, mc * 128:(mc + 1) * 128],
                                     ones_rp_bf, start=False, stop=True,
                                     skip_group_check=True)
                    nc.vector.tensor_copy(qT[:, mc * P:(mc + 1) * P], qps)
                # V[n, d'] = McT^T Wv (+bv)
                v_bf = acts.tile([128, NT * D], BF16, tag="vbf")
                for nt in range(NT):
                    vps = mm_psum.tile([128, D], F32, tag="mm")
                    for kc in range(DC):
                        nc.tensor.matmul(
                            vps,
                            mcT[:, kc * N + nt * 128: kc * N + nt * 128 + 128],
                            wv_bf[:, kc * D:(kc + 1) * D],
                            start=(kc == 0), stop=(kc == DC - 1))
                    nc.tensor.matmul(vps, ones_r128_bf, bv_row,
                                     start=False, stop=True,
                                     skip_group_check=True)
                    nc.vector.tensor_copy(v_bf[:, nt * D:(nt + 1) * D], vps)

                # scores2^T[n, p] = K^T(slice)^T @ Q^T ; softmax over n
                e2_sb = acts.tile([128, NT * P], BF16, tag="e2")
                den2_ps = mm_psum.tile([1, P], F32, tag="mm")
                for nt in range(NT):
                    s2ps = mm_psum.tile([128, P], F32, tag="mm")
                    for kc in range(DC):
                        nc.tensor.matmul(
                            s2ps,
                            kT[:, kc * N + nt * 128: kc * N + nt * 128 + 128],
                            qT[:, kc * P:(kc + 1) * P],
                            start=(kc == 0), stop=(kc == DC - 1))
                    nc.scalar.activation(e2_sb[:, nt * P:(nt + 1) * P], s2ps,
                                         ACTF.Exp,
                                         scale=1.0 / float(np.sqrt(D)))
                    nc.tensor.matmul(den2_ps, ones_c128_bf,
                                     e2_sb[:, nt * P:(nt + 1) * P],
                                     start=(nt == 0), stop=(nt == NT - 1))
                den2r = small.tile([1, P], F32, tag="den2r")
                nc.vector.reciprocal(den2r, den2_ps)
                d2rep_ps = mm_psum.tile([128, P], F32, tag="mm")
                nc.tensor.matmul(d2rep_ps, ones_r128_f32, den2r, start=True,
                                 stop=True)
                w2T = acts.tile([128, NT * P], BF16, tag="w2T")
                for nt in range(NT):
                    nc.vector.tensor_tensor(
                        w2T[:, nt * P:(nt + 1) * P],
                        e2_sb[:, nt * P:(nt + 1) * P], d2rep_ps, AX.mult)

                # attn_output[p, d] = w2T^T @ V   (output #1)
                ao_ps = mm_psum.tile([P, D], F32, tag="mm")
                for nt in range(NT):
                    nc.tensor.matmul(ao_ps, w2T[:, nt * P:(nt + 1) * P],
                                     v_bf[:, nt * D:(nt + 1) * D],
                                     start=(nt == 0), stop=(nt == NT - 1))
                ao_sb = acts.tile([P, D], F32, tag="aosb")
                nc.vector.tensor_copy(ao_sb, ao_ps)
                nc.sync.dma_start(attn_out[:, :], ao_sb)

                # attn_output^T[d, p] for the final projection
                aoT_ps = mm_psum.tile([128, DC * P], F32, tag="mm")
                for dc in range(DC):
                    for nt in range(NT):
                        nc.tensor.matmul(
                            aoT_ps[:, dc * P:(dc + 1) * P],
                            v_bf[:, nt * D + dc * 128: nt * D + (dc + 1) * 128],
                            w2T[:, nt * P:(nt + 1) * P],
                            start=(nt == 0), stop=(nt == NT - 1))
                aoT = acts.tile([128, DC * P], BF16, tag="aoT")
                nc.vector.tensor_copy(aoT, aoT_ps)

                # projected[p, a] = aoT^T @ Wfc + bfc, L2-normalize rows
                pr_ps = mm_psum.tile([P, ACTION_DIM], F32, tag="mm")
                for dc in range(DC):
                    nc.tensor.matmul(
                        pr_ps, aoT[:, dc * P:(dc + 1) * P],
                        wfc_bf[:, dc * ACTION_DIM:(dc + 1) * ACTION_DIM],
                        start=(dc == 0), stop=(dc == DC - 1))
                nc.tensor.matmul(pr_ps, ones_rp_bf, bfc_row,
                                 start=False, stop=True, skip_group_check=True)
                sq2 = tanh_pool.tile([P, ACTION_DIM], BF16, tag="l2sq")
                ss = small.tile([P, 1], F32, tag="l2ss")
                nc.scalar.activation(sq2, pr_ps, ACTF.Square, accum_out=ss)
                rn = _rsqrt(nc, small, ss, P)
                # 1/max(||x||, 1e-12) == min(rsqrt(ss), 1e12)
                nc.vector.tensor_scalar(rn, rn, 1e12, None, AX.min)
                pr_sb = acts.tile([P, ACTION_DIM], F32, tag="prsb")
                nc.scalar.activation(pr_sb, pr_ps, ACTF.Identity, scale=rn)
                nc.sync.dma_start(projected[:, :], pr_sb)
                mm_cm.__exit__(None, None, None)

            if loop_n is not None:
                # hint the PE back-edge target (body >256 PE instructions:
                # unhinted, each iteration stalls ~3-4us on IRAM refetch)
                with tc.For_i(0, loop_n, 1,
                              hint_engines=(mybir.EngineType.PE,)):
                    emit_body()
            else:
                for _rep in range(reps):
                    emit_body()

    nc.finalize()
    return nc


_CACHED_NC = {}


def _get_nc(reps=1, loop_n=None):
    key = (reps, loop_n)
    if key not in _CACHED_NC:
        _CACHED_NC[key] = build_nc(reps, loop_n)
    return _CACHED_NC[key]


def _make_in_maps(inputs):
    f = np.float32

    def arr(x):
        return np.ascontiguousarray(np.asarray(x, dtype=f))

    shared = {
        "W_alpha": arr(inputs["W_alpha"]), "U_alpha": arr(inputs["U_alpha"]),
        "Wq": arr(inputs["Wq"]), "Wk": arr(inputs["Wk"]), "Wv": arr(inputs["Wv"]),
        "Wf": arr(inputs["Wf"]), "Wfc": arr(inputs["Wfc"]),
        "b_alpha": arr(inputs["b_alpha"]).reshape(1, D),
        "bq": arr(inputs["bq"]).reshape(1, D),
        "bk": arr(inputs["bk"]).reshape(1, D),
        "bv": arr(inputs["bv"]).reshape(1, D),
        "bf": arr(inputs["bf"]).reshape(1, D),
        "ln_g": arr(inputs["ln_g"]).reshape(1, D),
        "ln_b": arr(inputs["ln_b"]).reshape(1, D),
        "bfc": arr(inputs["bfc"]).reshape(1, ACTION_DIM),
    }
    motion = arr(inputs["motion_features"])
    objf = arr(inputs["object_features"])
    return [
        {"motion": np.ascontiguousarray(motion[c]),
         "object": np.ascontiguousarray(objf[c]), **shared}
        for c in range(NC)
    ]


def _run(inputs, trace=False):
    nc = _get_nc()
    in_maps = _make_in_maps(inputs)
    res = run_bass_kernel_spmd(nc, in_maps, core_ids=list(range(NC)),
                               trace=trace)
    attn = np.stack([r["attn_out"] for r in res.results])
    proj = np.stack([r["projected"] for r in res.results])
    return (attn, proj), res


def kernel(**inputs):
    (attn, proj), _ = _run(inputs)
    return attn, proj


def bench(inputs, loops=(4, 36)):
    """Time the kernel body on device: build two NEFFs whose body runs in a
    hardware For_i loop loops[0] / loops[1] times, measure pipelined wall
    time for each, return the per-iteration slope in ns (cancels constant
    axon dispatch overhead)."""
    import time

    import jax
    from jax.experimental.shard_map import shard_map
    from jax.sharding import Mesh, PartitionSpec, NamedSharding
    import concourse.mybir as mb
    from concourse.bass2jax import _bass_exec_p, install_neuronx_cc_hook

    install_neuronx_cc_hook()
    in_maps = _make_in_maps(inputs)
    nc0 = _get_nc(1, loops[0])

    in_names, out_names, out_avals, zero_outs = [], [], [], []
    for alloc in nc0.m.functions[0].allocations:
        if not isinstance(alloc, mb.MemoryLocationSet):
            continue
        name = alloc.memorylocations[0].name
        if alloc.kind == "ExternalInput":
            in_names.append(name)
        elif alloc.kind == "ExternalOutput":
            shape = tuple(alloc.tensor_shape)
            dtype = mb.dt.np(alloc.dtype)
            out_names.append(name)
            out_avals.append(jax.core.ShapedArray(shape, dtype))
            zero_outs.append(np.zeros(shape, dtype))
    n_params = len(in_names)
    all_names = in_names + out_names

    devices = jax.devices()[:NC]
    mesh = Mesh(np.asarray(devices), ("core",))
    spec = PartitionSpec("core")
    in_specs = (spec,) * (n_params + len(out_names))
    out_specs = (spec,) * len(out_names)
    sharding = NamedSharding(mesh, spec)
    concat_in = [
        jax.device_put(
            np.concatenate([np.asarray(in_maps[c][n]) for c in range(NC)],
                           axis=0), sharding)
        for n in in_names
    ]
    concat_zero = [
        jax.device_put(np.zeros((NC * z.shape[0], *z.shape[1:]), z.dtype),
                       sharding)
        for z in zero_outs
    ]

    def make_fn(loop_n):
        nck = _get_nc(1, loop_n)

        def _bodyk(*args):
            outs = _bass_exec_p.bind(
                *args,
                out_avals=tuple(out_avals),
                in_names=tuple(all_names),
                out_names=tuple(out_names),
                lowering_input_output_aliases=(),
                sim_require_finite=True,
                sim_require_nnan=True,
                nc=nck,
            )
            return tuple(outs)

        fn = jax.jit(shard_map(_bodyk, mesh=mesh, in_specs=in_specs,
                               out_specs=out_specs, check_rep=False),
                     keep_unused=True)
        jax.block_until_ready(fn(*concat_in, *concat_zero))
        return fn

    fns = {k: make_fn(k) for k in loops}

    def timed(fn, iters=16):
        t0 = time.perf_counter()
        outs = [fn(*concat_in, *concat_zero) for _ in range(iters)]
        jax.block_until_ready(outs)
        return (time.perf_counter() - t0) / iters

    # interleave measurement rounds so slow drift cancels
    best = {k: None for k in loops}
    for _ in range(6):
        for k in loops:
            dt = timed(fns[k])
            best[k] = dt if best[k] is None else min(best[k], dt)
    k0, k1 = loops
    per_iter = (best[k1] - best[k0]) / (k1 - k0)
    print(f"bench: t{k0}={best[k0]*1e6:.1f}us  t{k1}={best[k1]*1e6:.1f}us  "
          f"slope={per_iter*1e6:.2f}us/iter")
    return per_iter * 1e9
